# revision 11
# baseline (speedup 1.0000x reference)
"""Trainium2 Bass kernel for GemNet AtomUpdateBlock (gnn_message_passing).

Computation (per reference):
    bases = basis_rad @ W_rbf              # [E, De]
    x     = m * bases                      # [E, De]
    z     = segment_sum(x, idx_atom, A)    # [A, De]
    x     = silu(z @ W_in)                 # [A, Da]
    3x residual: x = (x + silu(silu(x W1) W2)) / sqrt(2)

Distribution strategy: shard EDGES BY DESTINATION ATOM. Host bins the
20000 atoms into 8 cores x T_ATOM tiles of <=128 atoms (balanced by edge
count), sorts/pads each tile's edges into K 128-edge groups, and each
core computes the segment-sum + atom MLP for its own atoms only. No
collective needed at all; outputs are disjoint atom slices.

On-device per 128-edge tile:
    PE:  bases_psum = basis_radT_tile.T @ W_rbf   (K=16 matmul)
    DVE: x = bases_psum * m_tile (PSUM read), S = (iota == rel_idx)
    PE:  zT[c] += x[:,c*128:+128].T @ S  (one-hot scatter as matmul,
         accumulated in PSUM over the tile's K edge groups)
Epilogue per 128-atom tile (feature-major, activations [feat, atom]):
    ACT evacuates zT psum -> sbuf; dense1 + 3 residual layers as
    128x128-block matmuls; silu on ACT; skip-adds as fused
    scalar_tensor_tensor on DVE with host-folded sqrt2 scaling.
"""

import math
import os
import sys

import numpy as np

P = 128
N_CORES = 8
DE, DA, DR, NH = 512, 256, 16, 3
N_ATOMS, N_EDGES = 20000, 250000
T_ATOM = 20  # atom tiles per core (each up to 128 atoms)
INV_SQRT_2 = 0.7071067811865476

_NC_CACHE = {}
# Native Silu on the ACT engine (hardware). CoreSim doesn't implement Silu;
# tests flip this to False to emit Sigmoid + DVE multiply instead.
SILU_NATIVE = True


# ----------------------------------------------------------------------------
# Host-side packing
# ----------------------------------------------------------------------------

def _pack_layout(idx, n_atoms, n_cores, t_atom):
    """Assign atoms to (core, tile, slot) balancing edges; map edges to
    packed slots.

    Returns dict with per-edge and per-atom placement arrays and K (number
    of 128-edge groups per atom tile).
    """
    E = idx.shape[0]
    n_bins = n_cores * t_atom
    counts = np.bincount(idx, minlength=n_atoms)

    # Snake-deal atoms (sorted by edge count desc) into bins: balances both
    # edge totals and atom counts per bin.
    order = np.argsort(-counts, kind="stable")
    n_rounds = math.ceil(n_atoms / n_bins)
    pad = n_rounds * n_bins - n_atoms
    padded = np.concatenate([order, np.full(pad, -1, dtype=order.dtype)])
    grid = padded.reshape(n_rounds, n_bins)
    grid[1::2] = grid[1::2, ::-1]  # snake
    # grid[r, b] = atom placed in bin b at round r (or -1)
    bin_of_atom = np.empty(n_atoms, dtype=np.int64)
    slot_of_atom = np.empty(n_atoms, dtype=np.int64)
    valid = grid >= 0
    bin_idx = np.broadcast_to(np.arange(n_bins), grid.shape)
    round_idx = np.broadcast_to(np.arange(n_rounds)[:, None], grid.shape)
    bin_of_atom[grid[valid]] = bin_idx[valid]
    slot_of_atom[grid[valid]] = round_idx[valid]
    slots_per_bin = np.bincount(bin_of_atom, minlength=n_bins)
    assert slots_per_bin.max() <= P, "atom tile overflow"

    # Edges: sort by (bin, slot-of-atom) so each bin's edges are contiguous
    # and ordered by local atom slot.
    ebin = bin_of_atom[idx]
    eslot = slot_of_atom[idx]
    sort_key = ebin * (P + 1) + eslot
    eorder = np.argsort(sort_key, kind="stable")  # edge ids in packed order
    ebin_sorted = ebin[eorder]
    bin_counts = np.bincount(ebin_sorted, minlength=n_bins)
    K = max(1, math.ceil(bin_counts.max() / P))
    cap_bin = K * P
    bin_starts = np.zeros(n_bins + 1, dtype=np.int64)
    np.cumsum(bin_counts, out=bin_starts[1:])
    pos_in_bin = np.arange(E) - bin_starts[ebin_sorted]

    core_of_bin = np.arange(n_bins) // t_atom
    tile_of_bin = np.arange(n_bins) % t_atom
    # Packed flat slot within a core: tile*K*P + pos
    core_of_edge = core_of_bin[ebin_sorted]
    flat_slot = tile_of_bin[ebin_sorted] * cap_bin + pos_in_bin

    return dict(
        K=K,
        eorder=eorder,
        core_of_edge=core_of_edge,
        flat_slot=flat_slot,
        rel_of_edge=eslot[eorder].astype(np.float32),
        bin_of_atom=bin_of_atom,
        slot_of_atom=slot_of_atom,
        core_of_bin=core_of_bin,
        tile_of_bin=tile_of_bin,
    )


def _pack_weights(W_rbf, W_in, res_W1, res_W2):
    """Pack weights into lhsT block layouts.

    win_pack[p, (i*Cj+j)*P + q]  = W_in[i*P+p, j*P+q]
    wres_pack[p, fi*P + q]       = Wscaled[l][w][i*P+p, j*P+q],
        fi = ((l*2+w)*Cr+i)*Cr + j, W1 scaled by c^l (skip-chain folding).
    """
    Ci, Cj = DE // P, DA // P
    Cr = DA // P
    win = W_in.reshape(Ci, P, Cj, P).transpose(1, 0, 2, 3).reshape(P, Ci * Cj * P)
    blocks = []
    c = INV_SQRT_2
    for l in range(NH):
        w1 = (res_W1[l] * (c ** l)).astype(np.float32)
        w2 = res_W2[l].astype(np.float32)
        for w, W in ((0, w1), (1, w2)):
            blocks.append(
                W.reshape(Cr, P, Cr, P).transpose(1, 0, 2, 3).reshape(P, Cr * Cr * P)
            )
    wres = np.concatenate(blocks, axis=1)
    return (
        np.ascontiguousarray(W_rbf, dtype=np.float32),
        np.ascontiguousarray(win, dtype=np.float32),
        np.ascontiguousarray(wres, dtype=np.float32),
    )


def _build_in_maps(m, basis_rad, layout, W_rbf, W_in, res_W1, res_W2, n_cores, t_atom):
    K = layout["K"]
    cap = t_atom * K * P
    eorder = layout["eorder"]
    core_of_edge = layout["core_of_edge"]
    flat_slot = layout["flat_slot"]
    rel = layout["rel_of_edge"]

    wrbf, win, wres = _pack_weights(W_rbf, W_in, res_W1, res_W2)
    iota = np.broadcast_to(
        np.arange(P, dtype=np.float32), (P, P)
    ).copy()  # iota[p, j] = j

    m_src = m[eorder]
    bas_src = basis_rad[eorder]

    in_maps = []
    for c in range(n_cores):
        sel = core_of_edge == c
        fs = flat_slot[sel]
        m_pack = np.zeros((cap, DE), dtype=np.float32)
        m_pack[fs] = m_src[sel]
        basT = np.zeros((DR, cap), dtype=np.float32)
        basT[:, fs] = bas_src[sel].T
        rel_flat = np.full(cap, -1.0, dtype=np.float32)
        rel_flat[fs] = rel[sel]
        rel2 = np.ascontiguousarray(rel_flat.reshape(t_atom * K, P).T)
        in_maps.append(
            dict(
                m_pack=m_pack,
                basT=np.ascontiguousarray(basT),
                rel2=rel2,
                iota=iota,
                wrbf=wrbf,
                win=win,
                wres=wres,
            )
        )
    return in_maps


def _unpack_output(results, layout, n_atoms, n_cores, t_atom):
    """results: list of per-core out arrays [P, Cj*t_atom*P]."""
    Cj = DA // P
    out = np.zeros((n_atoms, DA), dtype=np.float32)
    bin_of_atom = layout["bin_of_atom"]
    slot_of_atom = layout["slot_of_atom"]
    core_of_bin = layout["core_of_bin"]
    tile_of_bin = layout["tile_of_bin"]
    core_of_atom = core_of_bin[bin_of_atom]
    row_of_atom = tile_of_bin[bin_of_atom] * P + slot_of_atom
    for c in range(n_cores):
        x = results[c]["out"].reshape(P, Cj, t_atom, P)
        # x[p, j, t, q] = result[t*P+q, j*P+p]
        x_core = x.transpose(2, 3, 1, 0).reshape(t_atom * P, DA)
        mask = core_of_atom == c
        out[mask] = x_core[row_of_atom[mask]]
    return out


# ----------------------------------------------------------------------------
# Bass kernel builder
# ----------------------------------------------------------------------------

def _build_nc(t_atom, K):
    import concourse.mybir as mybir
    import concourse.tile as tile
    from concourse import bacc

    f32 = mybir.dt.float32
    Ci, Cj = DE // P, DA // P
    Cr = DA // P
    cap = t_atom * K * P
    C3 = INV_SQRT_2 ** NH
    GAMMA = [float((1.0 / INV_SQRT_2) ** l) for l in range(NH)]

    nc = bacc.Bacc(
        "TRN2",
        target_bir_lowering=False,
        debug=False,
        enable_asserts=False,
        num_devices=N_CORES,
    )
    d_m = nc.dram_tensor("m_pack", [cap, DE], f32, kind="ExternalInput")
    d_basT = nc.dram_tensor("basT", [DR, cap], f32, kind="ExternalInput")
    d_rel = nc.dram_tensor("rel2", [P, t_atom * K], f32, kind="ExternalInput")
    d_iota = nc.dram_tensor("iota", [P, P], f32, kind="ExternalInput")
    d_wrbf = nc.dram_tensor("wrbf", [DR, DE], f32, kind="ExternalInput")
    d_win = nc.dram_tensor("win", [P, Ci * Cj * P], f32, kind="ExternalInput")
    d_wres = nc.dram_tensor("wres", [P, NH * 2 * Cr * Cr * P], f32, kind="ExternalInput")
    d_out = nc.dram_tensor("out", [P, Cj * t_atom * P], f32, kind="ExternalOutput")

    with tile.TileContext(nc) as tc:
        with (
            tc.tile_pool(name="const", bufs=1) as const_p,
            tc.tile_pool(name="bas", bufs=2) as bas_p,
            tc.tile_pool(name="m", bufs=4) as m_p,
            tc.tile_pool(name="x", bufs=3) as x_p,
            tc.tile_pool(name="s", bufs=3) as s_p,
            tc.tile_pool(name="ztsb", bufs=2) as ztsb_p,
            tc.tile_pool(name="act", bufs=3) as act_p,
            tc.tile_pool(name="outp", bufs=2) as out_p,
            tc.tile_pool(name="ps_bases", bufs=2, space="PSUM") as psb_p,
            tc.tile_pool(name="ps_zt", bufs=4, space="PSUM") as pszt_p,
            tc.tile_pool(name="ps_mlp", bufs=2, space="PSUM") as psmlp_p,
        ):
            _silu_ctr = [0]

            def emit_silu(out_ap, in_ps_ap):
                if SILU_NATIVE:
                    nc.scalar.activation(
                        out=out_ap, in_=in_ps_ap,
                        func=mybir.ActivationFunctionType.Silu,
                    )
                else:
                    _silu_ctr[0] += 1
                    sg = act_p.tile(
                        [P, DA], f32, tag="sig", name=f"sig{_silu_ctr[0]}"
                    )
                    nc.scalar.activation(
                        out=sg[:], in_=in_ps_ap,
                        func=mybir.ActivationFunctionType.Sigmoid,
                    )
                    nc.vector.tensor_tensor(
                        out=out_ap, in0=in_ps_ap, in1=sg[:],
                        op=mybir.AluOpType.mult,
                    )

            # Resident constants
            iota_sb = const_p.tile([P, P], f32, tag="iota")
            nc.sync.dma_start(out=iota_sb[:], in_=d_iota[:])
            rel_sb = const_p.tile([P, t_atom * K], f32, tag="rel")
            nc.sync.dma_start(out=rel_sb[:], in_=d_rel[:])
            wrbf_sb = const_p.tile([DR, DE], f32, tag="wrbf")
            nc.sync.dma_start(out=wrbf_sb[:], in_=d_wrbf[:])
            win_sb = const_p.tile([P, Ci * Cj * P], f32, tag="win")
            nc.sync.dma_start(out=win_sb[:], in_=d_win[:])
            wres_sb = const_p.tile([P, NH * 2 * Cr * Cr * P], f32, tag="wres")
            nc.sync.dma_start(out=wres_sb[:], in_=d_wres[:])

            for t in range(t_atom):
                # basis^T chunk for this atom tile: [16, K*P]
                bas_sb = bas_p.tile([DR, K * P], f32, tag="bas")
                nc.sync.dma_start(
                    out=bas_sb[:], in_=d_basT[:, t * K * P : (t + 1) * K * P]
                )
                zt_ps = [
                    pszt_p.tile([P, P], f32, space="PSUM", tag="zt", name=f"ztps{t}_{c}")
                    for c in range(Ci)
                ]
                for k in range(K):
                    col = t * K + k
                    m_t = m_p.tile([P, DE], f32, tag="m")
                    nc.sync.dma_start(
                        out=m_t[:], in_=d_m[col * P : (col + 1) * P, :]
                    )
                    b_ps = psb_p.tile([P, DE], f32, space="PSUM", tag="bases")
                    nc.tensor.matmul(
                        out=b_ps[:],
                        lhsT=bas_sb[:, k * P : (k + 1) * P],
                        rhs=wrbf_sb[:],
                        start=True,
                        stop=True,
                    )
                    x_t = x_p.tile([P, DE], f32, tag="x")
                    nc.vector.tensor_tensor(
                        out=x_t[:], in0=b_ps[:], in1=m_t[:], op=mybir.AluOpType.mult
                    )
                    s_t = s_p.tile([P, P], f32, tag="s")
                    nc.vector.tensor_scalar(
                        out=s_t[:],
                        in0=iota_sb[:],
                        scalar1=rel_sb[:, col : col + 1],
                        scalar2=None,
                        op0=mybir.AluOpType.is_equal,
                    )
                    for c in range(Ci):
                        nc.tensor.matmul(
                            out=zt_ps[c][:],
                            lhsT=x_t[:, c * P : (c + 1) * P],
                            rhs=s_t[:],
                            start=(k == 0),
                            stop=(k == K - 1),
                        )
                # ---- atom-tile epilogue (feature-major) ----
                zt_sb = ztsb_p.tile([P, DE], f32, tag="ztsb")
                for c in range(Ci):
                    nc.scalar.copy(out=zt_sb[:, c * P : (c + 1) * P], in_=zt_ps[c][:])
                u_ps = psmlp_p.tile([P, DA], f32, space="PSUM", tag="mlp")
                for j in range(Cj):
                    for c in range(Ci):
                        fi = c * Cj + j
                        nc.tensor.matmul(
                            out=u_ps[:, j * P : (j + 1) * P],
                            lhsT=win_sb[:, fi * P : (fi + 1) * P],
                            rhs=zt_sb[:, c * P : (c + 1) * P],
                            start=(c == 0),
                            stop=(c == Ci - 1),
                        )
                X = act_p.tile([P, DA], f32, tag="X")
                emit_silu(X[:], u_ps[:])
                for l in range(NH):
                    # u1 = silu(X @ W1s[l])
                    v_ps = psmlp_p.tile([P, DA], f32, space="PSUM", tag="mlp")
                    for j in range(Cr):
                        for i in range(Cr):
                            fi = ((l * 2 + 0) * Cr + i) * Cr + j
                            nc.tensor.matmul(
                                out=v_ps[:, j * P : (j + 1) * P],
                                lhsT=wres_sb[:, fi * P : (fi + 1) * P],
                                rhs=X[:, i * P : (i + 1) * P],
                                start=(i == 0),
                                stop=(i == Cr - 1),
                            )
                    u1 = act_p.tile([P, DA], f32, tag="u1")
                    emit_silu(u1[:], v_ps[:])
                    # y = silu(u1 @ W2[l])
                    w_ps = psmlp_p.tile([P, DA], f32, space="PSUM", tag="mlp")
                    for j in range(Cr):
                        for i in range(Cr):
                            fi = ((l * 2 + 1) * Cr + i) * Cr + j
                            nc.tensor.matmul(
                                out=w_ps[:, j * P : (j + 1) * P],
                                lhsT=wres_sb[:, fi * P : (fi + 1) * P],
                                rhs=u1[:, i * P : (i + 1) * P],
                                start=(i == 0),
                                stop=(i == Cr - 1),
                            )
                    y = act_p.tile([P, DA], f32, tag="y")
                    emit_silu(y[:], w_ps[:])
                    # X_{l+1} = X_l + gamma_l * y   (scale folded; DVE fused op)
                    Xn = act_p.tile([P, DA], f32, tag="X")
                    nc.vector.scalar_tensor_tensor(
                        out=Xn[:],
                        in0=y[:],
                        scalar=GAMMA[l],
                        in1=X[:],
                        op0=mybir.AluOpType.mult,
                        op1=mybir.AluOpType.add,
                    )
                    X = Xn
                # out = c^3 * X3  (scale+copy on ACT), then DMA out
                o_t = out_p.tile([P, DA], f32, tag="out")
                nc.scalar.mul(out=o_t[:], in_=X[:], mul=float(C3))
                for j in range(Cj):
                    nc.sync.dma_start(
                        out=d_out[:, (j * t_atom + t) * P : (j * t_atom + t + 1) * P],
                        in_=o_t[:, j * P : (j + 1) * P],
                    )
    nc.compile()
    return nc


def _get_nc(t_atom, K):
    key = (t_atom, K)
    if key not in _NC_CACHE:
        _NC_CACHE[key] = _build_nc(t_atom, K)
    return _NC_CACHE[key]


# ----------------------------------------------------------------------------
# Entry point
# ----------------------------------------------------------------------------

def kernel(h, m, basis_rad, idx_atom, W_rbf, W_in, res_W1, res_W2):
    from concourse.bass_utils import run_bass_kernel_spmd

    m = np.asarray(m, dtype=np.float32)
    basis_rad = np.asarray(basis_rad, dtype=np.float32)
    idx = np.asarray(idx_atom).astype(np.int64)
    W_rbf = np.asarray(W_rbf, dtype=np.float32)
    W_in = np.asarray(W_in, dtype=np.float32)
    res_W1 = np.asarray(res_W1, dtype=np.float32)
    res_W2 = np.asarray(res_W2, dtype=np.float32)
    n_atoms = np.asarray(h).shape[0]

    layout = _pack_layout(idx, n_atoms, N_CORES, T_ATOM)
    in_maps = _build_in_maps(
        m, basis_rad, layout, W_rbf, W_in, res_W1, res_W2, N_CORES, T_ATOM
    )
    nc = _get_nc(T_ATOM, layout["K"])

    trace = os.environ.get("KERNEL_TRACE", "0") == "1"
    res = run_bass_kernel_spmd(
        nc, in_maps, core_ids=list(range(N_CORES)), trace=trace
    )
    if trace and res.exec_time_ns is not None:
        print(f"HW exec time: {res.exec_time_ns} ns", file=sys.stderr)
        kernel.last_exec_time_ns = res.exec_time_ns
    kernel.last_results = res
    return _unpack_output(res.results, layout, n_atoms, N_CORES, T_ATOM)


# revision 12
# speedup vs baseline: 1.8920x; 1.8920x over previous
"""Trainium2 Bass kernel for GemNet AtomUpdateBlock (gnn_message_passing).

Computation (per reference):
    bases = basis_rad @ W_rbf              # [E, De]
    x     = m * bases                      # [E, De]
    z     = segment_sum(x, idx_atom, A)    # [A, De]
    x     = silu(z @ W_in)                 # [A, Da]
    3x residual: x = (x + silu(silu(x W1) W2)) / sqrt(2)

Distribution strategy: shard EDGES BY DESTINATION ATOM. The host bins the
atoms into 8 cores x T_ATOM tiles of <=128 atoms (balanced by edge count),
sorts/pads each tile's edges into K 128-edge groups, and each core computes
the segment-sum + atom MLP for its own atoms only. No collective needed;
outputs are disjoint atom slices.

Per 128-edge tile on device:
    PE (f32r):  bases_psum = basis_radT_tile.T @ W_rbf      (K=16, N=512)
    ACT:        evacuate bases cols [0,EVAC) psum->sbuf bf16
    DVE:        x[0:EVAC] = bases_sb * m (bf16 2x), x[EVAC:] = psum * m
    PE (bf16):  zT[c] += x[:,cP:+P].T @ S   (S = one-hot scatter matrix,
                precomputed host-side, DMA'd as bf16)
Epilogue per PAIR of 128-atom tiles (feature-major, f32r matmuls N=256):
    ACT evacuates zT psum -> sbuf f32r; dense1 + 3 residual layers as
    128-block matmuls; silu on ACT; skip-adds as one fused DVE
    scalar_tensor_tensor per layer with host-folded sqrt2 scaling.
"""

import math
import os
import sys

import numpy as np
import ml_dtypes

BF16 = ml_dtypes.bfloat16

P = 128
N_CORES = 8
DE, DA, DR, NH = 512, 256, 16, 3
T_ATOM = 20  # atom tiles per core (each up to 128 atoms); must be even
EVAC = 320  # bases columns evacuated via ACT (rest fused psum-mult on DVE)
INV_SQRT_2 = 0.7071067811865476

_NC_CACHE = {}
SILU_NATIVE = True


# ----------------------------------------------------------------------------
# Host-side packing
# ----------------------------------------------------------------------------

def _pack_layout(idx, n_atoms, n_cores, t_atom):
    E = idx.shape[0]
    n_bins = n_cores * t_atom
    counts = np.bincount(idx, minlength=n_atoms)

    order = np.argsort(-counts, kind="stable")
    n_rounds = math.ceil(n_atoms / n_bins)
    pad = n_rounds * n_bins - n_atoms
    padded = np.concatenate([order, np.full(pad, -1, dtype=order.dtype)])
    grid = padded.reshape(n_rounds, n_bins)
    grid[1::2] = grid[1::2, ::-1]  # snake-deal: balances edges and atoms
    bin_of_atom = np.empty(n_atoms, dtype=np.int64)
    slot_of_atom = np.empty(n_atoms, dtype=np.int64)
    valid = grid >= 0
    bin_idx = np.broadcast_to(np.arange(n_bins), grid.shape)
    round_idx = np.broadcast_to(np.arange(n_rounds)[:, None], grid.shape)
    bin_of_atom[grid[valid]] = bin_idx[valid]
    slot_of_atom[grid[valid]] = round_idx[valid]
    assert np.bincount(bin_of_atom, minlength=n_bins).max() <= P

    ebin = bin_of_atom[idx]
    eslot = slot_of_atom[idx]
    eorder = np.argsort(ebin * (P + 1) + eslot, kind="stable")
    ebin_sorted = ebin[eorder]
    bin_counts = np.bincount(ebin_sorted, minlength=n_bins)
    K = max(1, math.ceil(bin_counts.max() / P))
    bin_starts = np.zeros(n_bins + 1, dtype=np.int64)
    np.cumsum(bin_counts, out=bin_starts[1:])
    pos_in_bin = np.arange(E) - bin_starts[ebin_sorted]

    core_of_bin = np.arange(n_bins) // t_atom
    tile_of_bin = np.arange(n_bins) % t_atom
    return dict(
        K=K,
        eorder=eorder,
        core_of_edge=core_of_bin[ebin_sorted],
        flat_slot=tile_of_bin[ebin_sorted] * (K * P) + pos_in_bin,
        rel_of_edge=eslot[eorder].astype(np.int64),
        bin_of_atom=bin_of_atom,
        slot_of_atom=slot_of_atom,
        core_of_bin=core_of_bin,
        tile_of_bin=tile_of_bin,
    )


def _pack_weights(W_rbf, W_in, res_W1, res_W2):
    Ci, Cj = DE // P, DA // P
    Cr = DA // P
    win = W_in.reshape(Ci, P, Cj, P).transpose(1, 0, 2, 3).reshape(P, Ci * Cj * P)
    blocks = []
    c = INV_SQRT_2
    for l in range(NH):
        w1 = (res_W1[l] * (c ** l)).astype(np.float32)
        w2 = res_W2[l].astype(np.float32)
        for W in (w1, w2):
            blocks.append(
                W.reshape(Cr, P, Cr, P).transpose(1, 0, 2, 3).reshape(P, Cr * Cr * P)
            )
    wres = np.concatenate(blocks, axis=1)
    return (
        np.ascontiguousarray(W_rbf, dtype=np.float32),
        np.ascontiguousarray(win, dtype=np.float32),
        np.ascontiguousarray(wres, dtype=np.float32),
    )


def _build_in_maps(m, basis_rad, layout, W_rbf, W_in, res_W1, res_W2, n_cores, t_atom):
    K = layout["K"]
    cap = t_atom * K * P
    ncols = t_atom * K
    eorder = layout["eorder"]
    core_of_edge = layout["core_of_edge"]
    flat_slot = layout["flat_slot"]
    rel = layout["rel_of_edge"]

    wrbf, win, wres = _pack_weights(W_rbf, W_in, res_W1, res_W2)
    m_src = m[eorder]
    bas_src = basis_rad[eorder]

    in_maps = []
    for c in range(n_cores):
        sel = core_of_edge == c
        fs = flat_slot[sel]
        m_pack = np.zeros((cap, DE), dtype=BF16)
        m_pack[fs] = m_src[sel].astype(BF16)
        basT = np.zeros((DR, cap), dtype=np.float32)
        basT[:, fs] = bas_src[sel].T
        # One-hot scatter matrix S[p, col, a] = (rel(edge col*P+p) == a), bf16
        rel_flat = np.full(cap, -1, dtype=np.int64)
        rel_flat[fs] = rel[sel]
        rel2 = rel_flat.reshape(ncols, P).T  # [p, col]
        s_host = (rel2[:, :, None] == np.arange(P)[None, None, :]).astype(BF16)
        in_maps.append(
            dict(
                m_pack=m_pack,
                basT=np.ascontiguousarray(basT),
                s_hot=np.ascontiguousarray(s_host.reshape(P, ncols * P)),
                wrbf=wrbf,
                win=win,
                wres=wres,
            )
        )
    return in_maps


def _unpack_output(results, layout, n_atoms, n_cores, t_atom):
    Cj = DA // P
    out = np.zeros((n_atoms, DA), dtype=np.float32)
    core_of_atom = layout["core_of_bin"][layout["bin_of_atom"]]
    row_of_atom = (
        layout["tile_of_bin"][layout["bin_of_atom"]] * P + layout["slot_of_atom"]
    )
    for c in range(n_cores):
        x = results[c]["out"].reshape(P, Cj, t_atom, P)
        x_core = x.transpose(2, 3, 1, 0).reshape(t_atom * P, DA)
        mask = core_of_atom == c
        out[mask] = x_core[row_of_atom[mask]]
    return out


# ----------------------------------------------------------------------------
# Bass kernel builder
# ----------------------------------------------------------------------------

def _build_nc(t_atom, K):
    import concourse.mybir as mybir
    import concourse.tile as tile
    from concourse import bacc

    f32 = mybir.dt.float32
    f32r = mybir.dt.float32r
    bf16 = mybir.dt.bfloat16
    Ci, Cj = DE // P, DA // P
    Cr = DA // P
    cap = t_atom * K * P
    ncols = t_atom * K
    C3 = INV_SQRT_2 ** NH
    GAMMA = [float((1.0 / INV_SQRT_2) ** l) for l in range(NH)]
    assert t_atom % 2 == 0
    n_pairs = t_atom // 2
    W2 = 2 * P  # atoms per epilogue pair

    nc = bacc.Bacc(
        "TRN2",
        target_bir_lowering=False,
        debug=False,
        enable_asserts=False,
        num_devices=N_CORES,
    )
    d_m = nc.dram_tensor("m_pack", [cap, DE], bf16, kind="ExternalInput")
    d_basT = nc.dram_tensor("basT", [DR, cap], f32r, kind="ExternalInput")
    d_s = nc.dram_tensor("s_hot", [P, ncols * P], bf16, kind="ExternalInput")
    d_wrbf = nc.dram_tensor("wrbf", [DR, DE], f32r, kind="ExternalInput")
    d_win = nc.dram_tensor("win", [P, Ci * Cj * P], f32r, kind="ExternalInput")
    d_wres = nc.dram_tensor(
        "wres", [P, NH * 2 * Cr * Cr * P], f32r, kind="ExternalInput"
    )
    d_out = nc.dram_tensor("out", [P, Cj * t_atom * P], f32, kind="ExternalOutput")

    with tile.TileContext(nc) as tc:
        with (
            tc.tile_pool(name="const", bufs=1) as const_p,
            tc.tile_pool(name="bas", bufs=2) as bas_p,
            tc.tile_pool(name="m", bufs=4) as m_p,
            tc.tile_pool(name="x", bufs=3) as x_p,
            tc.tile_pool(name="s", bufs=4) as s_p,
            tc.tile_pool(name="blo", bufs=3) as blo_p,
            tc.tile_pool(name="ztsb", bufs=2) as ztsb_p,
            tc.tile_pool(name="act", bufs=3) as act_p,
            tc.tile_pool(name="outp", bufs=2) as out_p,
            tc.tile_pool(name="ps_bases", bufs=2, space="PSUM") as psb_p,
            tc.tile_pool(name="ps_zt", bufs=4, space="PSUM") as pszt_p,
            tc.tile_pool(name="ps_mlp", bufs=2, space="PSUM") as psmlp_p,
        ):
            _ctr = [0]

            def emit_silu(out_ap, in_ps_ap):
                if SILU_NATIVE:
                    nc.scalar.activation(
                        out=out_ap, in_=in_ps_ap,
                        func=mybir.ActivationFunctionType.Silu,
                    )
                else:
                    _ctr[0] += 1
                    sg = act_p.tile(
                        [P, W2], f32, tag="sig", name=f"sig{_ctr[0]}"
                    )
                    nc.scalar.activation(
                        out=sg[:], in_=in_ps_ap,
                        func=mybir.ActivationFunctionType.Sigmoid,
                    )
                    nc.vector.tensor_tensor(
                        out=out_ap, in0=in_ps_ap, in1=sg[:],
                        op=mybir.AluOpType.mult,
                    )

            # Resident constants
            wrbf_sb = const_p.tile([DR, DE], f32r, tag="wrbf")
            nc.sync.dma_start(out=wrbf_sb[:], in_=d_wrbf[:])
            win_sb = const_p.tile([P, Ci * Cj * P], f32r, tag="win")
            nc.sync.dma_start(out=win_sb[:], in_=d_win[:])
            wres_sb = const_p.tile([P, NH * 2 * Cr * Cr * P], f32r, tag="wres")
            nc.sync.dma_start(out=wres_sb[:], in_=d_wres[:])

            for g in range(n_pairs):
                zt_sub = []  # zt psum tiles for the two subtiles
                for sub in range(2):
                    t = 2 * g + sub
                    bas_sb = bas_p.tile([DR, K * P], f32r, tag="bas")
                    nc.sync.dma_start(
                        out=bas_sb[:], in_=d_basT[:, t * K * P : (t + 1) * K * P]
                    )
                    zt_ps = [
                        pszt_p.tile(
                            [P, P], f32, space="PSUM", tag="zt", name=f"ztps{t}_{c}"
                        )
                        for c in range(Ci)
                    ]
                    zt_sub.append(zt_ps)
                    for k in range(K):
                        col = t * K + k
                        m_t = m_p.tile([P, DE], bf16, tag="m")
                        nc.sync.dma_start(
                            out=m_t[:], in_=d_m[col * P : (col + 1) * P, :]
                        )
                        s_t = s_p.tile([P, P], bf16, tag="s")
                        nc.sync.dma_start(
                            out=s_t[:], in_=d_s[:, col * P : (col + 1) * P]
                        )
                        b_ps = psb_p.tile([P, DE], f32, space="PSUM", tag="bases")
                        nc.tensor.matmul(
                            out=b_ps[:],
                            lhsT=bas_sb[:, k * P : (k + 1) * P],
                            rhs=wrbf_sb[:],
                            start=True,
                            stop=True,
                        )
                        x_t = x_p.tile([P, DE], bf16, tag="x")
                        # ACT evacuates [0, EVAC) to sbuf bf16; DVE multiplies
                        blo = blo_p.tile([P, EVAC], bf16, tag="blo")
                        nc.scalar.copy(out=blo[:], in_=b_ps[:, 0:EVAC])
                        nc.vector.tensor_tensor(
                            out=x_t[:, 0:EVAC],
                            in0=blo[:],
                            in1=m_t[:, 0:EVAC],
                            op=mybir.AluOpType.mult,
                        )
                        # DVE fused psum-read multiply for the tail columns
                        nc.vector.tensor_tensor(
                            out=x_t[:, EVAC:DE],
                            in0=b_ps[:, EVAC:DE],
                            in1=m_t[:, EVAC:DE],
                            op=mybir.AluOpType.mult,
                        )
                        for c in range(Ci):
                            nc.tensor.matmul(
                                out=zt_ps[c][:],
                                lhsT=x_t[:, c * P : (c + 1) * P],
                                rhs=s_t[:],
                                start=(k == 0),
                                stop=(k == K - 1),
                            )
                # ---- paired epilogue (256 atoms, feature-major, f32r) ----
                zt_sb = ztsb_p.tile([P, Ci * W2], f32r, tag="ztsb")
                for sub in range(2):
                    for c in range(Ci):
                        nc.scalar.copy(
                            out=zt_sb[:, c * W2 + sub * P : c * W2 + sub * P + P],
                            in_=zt_sub[sub][c][:],
                        )
                u_ps = [
                    psmlp_p.tile(
                        [P, W2], f32, space="PSUM", tag="mlp", name=f"ups{g}_{j}"
                    )
                    for j in range(Cj)
                ]
                for j in range(Cj):
                    for c in range(Ci):
                        fi = c * Cj + j
                        nc.tensor.matmul(
                            out=u_ps[j][:],
                            lhsT=win_sb[:, fi * P : (fi + 1) * P],
                            rhs=zt_sb[:, c * W2 : (c + 1) * W2],
                            start=(c == 0),
                            stop=(c == Ci - 1),
                        )
                X = act_p.tile([P, Cr * W2], f32r, tag="X", name=f"X{g}_0")
                for j in range(Cj):
                    emit_silu(X[:, j * W2 : (j + 1) * W2], u_ps[j][:])
                for l in range(NH):
                    v_ps = [
                        psmlp_p.tile(
                            [P, W2], f32, space="PSUM", tag="mlp", name=f"vps{g}_{l}_{j}"
                        )
                        for j in range(Cr)
                    ]
                    for j in range(Cr):
                        for i in range(Cr):
                            fi = ((l * 2 + 0) * Cr + i) * Cr + j
                            nc.tensor.matmul(
                                out=v_ps[j][:],
                                lhsT=wres_sb[:, fi * P : (fi + 1) * P],
                                rhs=X[:, i * W2 : (i + 1) * W2],
                                start=(i == 0),
                                stop=(i == Cr - 1),
                            )
                    u1 = act_p.tile([P, Cr * W2], f32r, tag="u1", name=f"u1_{g}_{l}")
                    for j in range(Cr):
                        emit_silu(u1[:, j * W2 : (j + 1) * W2], v_ps[j][:])
                    w_ps = [
                        psmlp_p.tile(
                            [P, W2], f32, space="PSUM", tag="mlp", name=f"wps{g}_{l}_{j}"
                        )
                        for j in range(Cr)
                    ]
                    for j in range(Cr):
                        for i in range(Cr):
                            fi = ((l * 2 + 1) * Cr + i) * Cr + j
                            nc.tensor.matmul(
                                out=w_ps[j][:],
                                lhsT=wres_sb[:, fi * P : (fi + 1) * P],
                                rhs=u1[:, i * W2 : (i + 1) * W2],
                                start=(i == 0),
                                stop=(i == Cr - 1),
                            )
                    Y = act_p.tile([P, Cr * W2], f32r, tag="y", name=f"Y{g}_{l}")
                    for j in range(Cr):
                        emit_silu(Y[:, j * W2 : (j + 1) * W2], w_ps[j][:])
                    Xn = act_p.tile([P, Cr * W2], f32r, tag="X", name=f"X{g}_{l + 1}")
                    nc.vector.scalar_tensor_tensor(
                        out=Xn[:],
                        in0=Y[:],
                        scalar=GAMMA[l],
                        in1=X[:],
                        op0=mybir.AluOpType.mult,
                        op1=mybir.AluOpType.add,
                    )
                    X = Xn
                o_t = out_p.tile([P, Cj * W2], f32, tag="out")
                nc.scalar.mul(out=o_t[:], in_=X[:], mul=float(C3))
                for j in range(Cj):
                    nc.sync.dma_start(
                        out=d_out[:, (j * t_atom + 2 * g) * P : (j * t_atom + 2 * g + 2) * P],
                        in_=o_t[:, j * W2 : (j + 1) * W2],
                    )
    nc.compile()
    return nc


def _get_nc(t_atom, K):
    key = (t_atom, K)
    if key not in _NC_CACHE:
        _NC_CACHE[key] = _build_nc(t_atom, K)
    return _NC_CACHE[key]


# ----------------------------------------------------------------------------
# Entry point
# ----------------------------------------------------------------------------

def kernel(h, m, basis_rad, idx_atom, W_rbf, W_in, res_W1, res_W2):
    from concourse.bass_utils import run_bass_kernel_spmd

    m = np.asarray(m, dtype=np.float32)
    basis_rad = np.asarray(basis_rad, dtype=np.float32)
    idx = np.asarray(idx_atom).astype(np.int64)
    W_rbf = np.asarray(W_rbf, dtype=np.float32)
    W_in = np.asarray(W_in, dtype=np.float32)
    res_W1 = np.asarray(res_W1, dtype=np.float32)
    res_W2 = np.asarray(res_W2, dtype=np.float32)
    n_atoms = np.asarray(h).shape[0]

    layout = _pack_layout(idx, n_atoms, N_CORES, T_ATOM)
    in_maps = _build_in_maps(
        m, basis_rad, layout, W_rbf, W_in, res_W1, res_W2, N_CORES, T_ATOM
    )
    nc = _get_nc(T_ATOM, layout["K"])

    trace = os.environ.get("KERNEL_TRACE", "0") == "1"
    res = run_bass_kernel_spmd(
        nc, in_maps, core_ids=list(range(N_CORES)), trace=trace
    )
    if trace and res.exec_time_ns is not None:
        print(f"HW exec time: {res.exec_time_ns} ns", file=sys.stderr)
        kernel.last_exec_time_ns = res.exec_time_ns
    kernel.last_results = res
    return _unpack_output(res.results, layout, n_atoms, N_CORES, T_ATOM)


# revision 14
# speedup vs baseline: 2.5614x; 1.3538x over previous
"""Trainium2 Bass kernel for GemNet AtomUpdateBlock (gnn_message_passing).

Computation (per reference):
    bases = basis_rad @ W_rbf              # [E, De]
    x     = m * bases                      # [E, De]
    z     = segment_sum(x, idx_atom, A)    # [A, De]
    x     = silu(z @ W_in)                 # [A, Da]
    3x residual: x = (x + silu(silu(x W1) W2)) / sqrt(2)

Distribution strategy: shard EDGES BY DESTINATION ATOM. The host bins the
atoms into 8 cores x T_ATOM tiles of <=128 atoms (balanced by edge count),
sorts/pads each tile's edges into K 128-edge groups, and each core computes
the segment-sum + atom MLP for its own atoms only. No collective needed;
outputs are disjoint atom slices.

Per 128-edge tile on device (bf16 matmuls, f32 PSUM):
    PE:  bases_psum = basis_radT_tile.T @ W_rbf     (K=16, N=512)
    ACT: evacuate bases cols [0,EVAC) psum->sbuf bf16
    DVE: x[0:EVAC] = bases_sb * m (bf16 2x), x[EVAC:] = psum * m
    PE:  z[a,:] += S.T @ x  (one matmul, N=512; S = one-hot scatter matrix
         precomputed host-side, DMA'd bf16, loaded as PE weights)
Epilogue per PAIR of 128-atom tiles: z evac -> 4 PE transposes (f32) to
feature-major zT -> bf16 MLP matmuls N=256, silu on ACT, skip-adds as one
fused DVE scalar_tensor_tensor per layer with host-folded sqrt2 scaling.
"""

import math
import os
import sys

import numpy as np
import ml_dtypes

BF16 = ml_dtypes.bfloat16

P = 128
N_CORES = 8
DE, DA, DR, NH = 512, 256, 16, 3
T_ATOM = 20  # atom tiles per core (each up to 128 atoms); must be even
EVAC = 256  # bases columns evacuated via ACT (rest fused psum-mult on DVE)
INV_SQRT_2 = 0.7071067811865476

_NC_CACHE = {}
SILU_NATIVE = True


# ----------------------------------------------------------------------------
# Host-side packing
# ----------------------------------------------------------------------------

def _pack_layout(idx, n_atoms, n_cores, t_atom):
    E = idx.shape[0]
    n_bins = n_cores * t_atom
    counts = np.bincount(idx, minlength=n_atoms)

    order = np.argsort(-counts, kind="stable")
    n_rounds = math.ceil(n_atoms / n_bins)
    pad = n_rounds * n_bins - n_atoms
    padded = np.concatenate([order, np.full(pad, -1, dtype=order.dtype)])
    grid = padded.reshape(n_rounds, n_bins)
    grid[1::2] = grid[1::2, ::-1]  # snake-deal: balances edges and atoms
    bin_of_atom = np.empty(n_atoms, dtype=np.int64)
    slot_of_atom = np.empty(n_atoms, dtype=np.int64)
    valid = grid >= 0
    bin_idx = np.broadcast_to(np.arange(n_bins), grid.shape)
    round_idx = np.broadcast_to(np.arange(n_rounds)[:, None], grid.shape)
    bin_of_atom[grid[valid]] = bin_idx[valid]
    slot_of_atom[grid[valid]] = round_idx[valid]
    assert np.bincount(bin_of_atom, minlength=n_bins).max() <= P

    ebin = bin_of_atom[idx]
    eslot = slot_of_atom[idx]
    eorder = np.argsort(ebin * (P + 1) + eslot, kind="stable")
    ebin_sorted = ebin[eorder]
    bin_counts = np.bincount(ebin_sorted, minlength=n_bins)
    K = max(1, math.ceil(bin_counts.max() / P))
    bin_starts = np.zeros(n_bins + 1, dtype=np.int64)
    np.cumsum(bin_counts, out=bin_starts[1:])
    pos_in_bin = np.arange(E) - bin_starts[ebin_sorted]

    core_of_bin = np.arange(n_bins) // t_atom
    tile_of_bin = np.arange(n_bins) % t_atom
    return dict(
        K=K,
        eorder=eorder,
        core_of_edge=core_of_bin[ebin_sorted],
        flat_slot=tile_of_bin[ebin_sorted] * (K * P) + pos_in_bin,
        rel_of_edge=eslot[eorder].astype(np.int64),
        bin_of_atom=bin_of_atom,
        slot_of_atom=slot_of_atom,
        core_of_bin=core_of_bin,
        tile_of_bin=tile_of_bin,
    )


def _pack_weights(W_rbf, W_in, res_W1, res_W2):
    Ci, Cj = DE // P, DA // P
    Cr = DA // P
    win = W_in.reshape(Ci, P, Cj, P).transpose(1, 0, 2, 3).reshape(P, Ci * Cj * P)
    blocks = []
    c = INV_SQRT_2
    for l in range(NH):
        w1 = (res_W1[l] * (c ** l)).astype(np.float32)
        w2 = res_W2[l].astype(np.float32)
        for W in (w1, w2):
            blocks.append(
                W.reshape(Cr, P, Cr, P).transpose(1, 0, 2, 3).reshape(P, Cr * Cr * P)
            )
    wres = np.concatenate(blocks, axis=1)
    return (
        np.ascontiguousarray(W_rbf, dtype=BF16),
        np.ascontiguousarray(win, dtype=BF16),
        np.ascontiguousarray(wres, dtype=BF16),
    )


def _build_in_maps(m, basis_rad, layout, W_rbf, W_in, res_W1, res_W2, n_cores, t_atom):
    K = layout["K"]
    cap = t_atom * K * P
    ncols = t_atom * K
    eorder = layout["eorder"]
    core_of_edge = layout["core_of_edge"]
    flat_slot = layout["flat_slot"]
    rel = layout["rel_of_edge"]

    wrbf, win, wres = _pack_weights(W_rbf, W_in, res_W1, res_W2)
    m_src = m[eorder]
    bas_src = basis_rad[eorder]

    in_maps = []
    for c in range(n_cores):
        sel = core_of_edge == c
        fs = flat_slot[sel]
        m_pack = np.zeros((cap, DE), dtype=BF16)
        m_pack[fs] = m_src[sel].astype(BF16)
        basT = np.zeros((DR, cap), dtype=BF16)
        basT[:, fs] = bas_src[sel].T.astype(BF16)
        rel_flat = np.full(cap, -1, dtype=np.int64)
        rel_flat[fs] = rel[sel]
        rel2 = rel_flat.reshape(ncols, P).T  # [p, col]
        s_host = (rel2[:, :, None] == np.arange(P)[None, None, :]).astype(BF16)
        in_maps.append(
            dict(
                m_pack=m_pack,
                basT=np.ascontiguousarray(basT),
                s_hot=np.ascontiguousarray(s_host.reshape(P, ncols * P)),
                wrbf=wrbf,
                win=win,
                wres=wres,
            )
        )
    return in_maps


def _unpack_output(results, layout, n_atoms, n_cores, t_atom):
    Cj = DA // P
    out = np.zeros((n_atoms, DA), dtype=np.float32)
    core_of_atom = layout["core_of_bin"][layout["bin_of_atom"]]
    row_of_atom = (
        layout["tile_of_bin"][layout["bin_of_atom"]] * P + layout["slot_of_atom"]
    )
    for c in range(n_cores):
        x = results[c]["out"].reshape(P, Cj, t_atom, P)
        x_core = x.transpose(2, 3, 1, 0).reshape(t_atom * P, DA)
        mask = core_of_atom == c
        out[mask] = x_core[row_of_atom[mask]]
    return out


# ----------------------------------------------------------------------------
# Bass kernel builder
# ----------------------------------------------------------------------------

def _build_nc(t_atom, K):
    import concourse.mybir as mybir
    import concourse.tile as tile
    from concourse import bacc
    from concourse.masks import make_identity

    f32 = mybir.dt.float32
    bf16 = mybir.dt.bfloat16
    Ci, Cj = DE // P, DA // P
    Cr = DA // P
    cap = t_atom * K * P
    ncols = t_atom * K
    C3 = INV_SQRT_2 ** NH
    GAMMA = [float((1.0 / INV_SQRT_2) ** l) for l in range(NH)]
    assert t_atom % 2 == 0
    n_pairs = t_atom // 2
    W2 = 2 * P  # atoms per epilogue pair

    nc = bacc.Bacc(
        "TRN2",
        target_bir_lowering=False,
        debug=False,
        enable_asserts=False,
        num_devices=N_CORES,
    )
    d_m = nc.dram_tensor("m_pack", [cap, DE], bf16, kind="ExternalInput")
    d_basT = nc.dram_tensor("basT", [DR, cap], bf16, kind="ExternalInput")
    d_s = nc.dram_tensor("s_hot", [P, ncols * P], bf16, kind="ExternalInput")
    d_wrbf = nc.dram_tensor("wrbf", [DR, DE], bf16, kind="ExternalInput")
    d_win = nc.dram_tensor("win", [P, Ci * Cj * P], bf16, kind="ExternalInput")
    d_wres = nc.dram_tensor(
        "wres", [P, NH * 2 * Cr * Cr * P], bf16, kind="ExternalInput"
    )
    d_out = nc.dram_tensor("out", [P, Cj * t_atom * P], f32, kind="ExternalOutput")

    with tile.TileContext(nc) as tc:
        with (
            tc.tile_pool(name="const", bufs=1) as const_p,
            tc.tile_pool(name="bas", bufs=2) as bas_p,
            tc.tile_pool(name="m", bufs=2) as m_p,
            tc.tile_pool(name="x", bufs=3) as x_p,
            tc.tile_pool(name="s", bufs=2) as s_p,
            tc.tile_pool(name="blo", bufs=3) as blo_p,
            tc.tile_pool(name="zsb", bufs=2) as zsb_p,
            tc.tile_pool(name="ztsb", bufs=2) as ztsb_p,
            tc.tile_pool(name="act", bufs=3) as act_p,
            tc.tile_pool(name="outp", bufs=2) as out_p,
            tc.tile_pool(name="ps_bases", bufs=2, space="PSUM") as psb_p,
            tc.tile_pool(name="ps_z", bufs=2, space="PSUM") as psz_p,
            tc.tile_pool(name="ps_misc", bufs=4, space="PSUM") as psm_p,
        ):
            _ctr = [0]

            def emit_silu(out_ap, in_ps_ap):
                if SILU_NATIVE:
                    nc.scalar.activation(
                        out=out_ap, in_=in_ps_ap,
                        func=mybir.ActivationFunctionType.Silu,
                    )
                else:
                    _ctr[0] += 1
                    sg = act_p.tile([P, W2], f32, tag="sig", name=f"sig{_ctr[0]}")
                    nc.scalar.activation(
                        out=sg[:], in_=in_ps_ap,
                        func=mybir.ActivationFunctionType.Sigmoid,
                    )
                    nc.vector.tensor_tensor(
                        out=out_ap, in0=in_ps_ap, in1=sg[:],
                        op=mybir.AluOpType.mult,
                    )

            # Resident constants
            wrbf_sb = const_p.tile([DR, DE], bf16, tag="wrbf")
            nc.sync.dma_start(out=wrbf_sb[:], in_=d_wrbf[:])
            win_sb = const_p.tile([P, Ci * Cj * P], bf16, tag="win")
            nc.sync.dma_start(out=win_sb[:], in_=d_win[:])
            wres_sb = const_p.tile([P, NH * 2 * Cr * Cr * P], bf16, tag="wres")
            nc.sync.dma_start(out=wres_sb[:], in_=d_wres[:])
            ident = const_p.tile([P, P], f32, tag="ident")
            make_identity(nc, ident[:])

            for g in range(n_pairs):
                z_sub = []
                for sub in range(2):
                    t = 2 * g + sub
                    bas_sb = bas_p.tile([DR, K * P], bf16, tag="bas")
                    nc.sync.dma_start(
                        out=bas_sb[:], in_=d_basT[:, t * K * P : (t + 1) * K * P]
                    )
                    # whole atom tile's m and S in one DMA each
                    m_t = m_p.tile([P, K * DE], bf16, tag="m")
                    nc.sync.dma_start(
                        out=m_t[:].rearrange("p (k d) -> p k d", k=K),
                        in_=d_m[t * K * P : (t + 1) * K * P, :].rearrange(
                            "(k p) d -> p k d", p=P
                        ),
                    )
                    s_t = s_p.tile([P, K * P], bf16, tag="s")
                    nc.sync.dma_start(
                        out=s_t[:], in_=d_s[:, t * K * P : (t + 1) * K * P]
                    )
                    z_ps = psz_p.tile(
                        [P, DE], f32, space="PSUM", tag="z", name=f"zps{t}"
                    )
                    z_sub.append(z_ps)
                    for k in range(K):
                        b_ps = psb_p.tile([P, DE], f32, space="PSUM", tag="bases")
                        nc.tensor.matmul(
                            out=b_ps[:],
                            lhsT=bas_sb[:, k * P : (k + 1) * P],
                            rhs=wrbf_sb[:],
                            start=True,
                            stop=True,
                        )
                        x_t = x_p.tile([P, DE], bf16, tag="x")
                        blo = blo_p.tile([P, EVAC], bf16, tag="blo")
                        nc.scalar.copy(out=blo[:], in_=b_ps[:, 0:EVAC])
                        nc.vector.tensor_tensor(
                            out=x_t[:, 0:EVAC],
                            in0=blo[:],
                            in1=m_t[:, k * DE : k * DE + EVAC],
                            op=mybir.AluOpType.mult,
                        )
                        nc.vector.tensor_tensor(
                            out=x_t[:, EVAC:DE],
                            in0=b_ps[:, EVAC:DE],
                            in1=m_t[:, k * DE + EVAC : (k + 1) * DE],
                            op=mybir.AluOpType.mult,
                        )
                        nc.tensor.matmul(
                            out=z_ps[:],
                            lhsT=s_t[:, k * P : (k + 1) * P],
                            rhs=x_t[:],
                            start=(k == 0),
                            stop=(k == K - 1),
                        )
                # ---- paired epilogue (256 atoms, feature-major, bf16) ----
                zt_sb = ztsb_p.tile([P, Ci * W2], bf16, tag="ztsb")
                for sub in range(2):
                    t = 2 * g + sub
                    z_sb = zsb_p.tile([P, DE], f32, tag="zsb", name=f"zsb{t}")
                    nc.scalar.copy(out=z_sb[:], in_=z_sub[sub][:])
                    for c in range(Ci):
                        zt_ps = psm_p.tile(
                            [P, P], f32, space="PSUM", tag="misc", name=f"ztp{t}_{c}"
                        )
                        nc.tensor.transpose(
                            out=zt_ps[:],
                            in_=z_sb[:, c * P : (c + 1) * P],
                            identity=ident[:],
                        )
                        nc.scalar.copy(
                            out=zt_sb[:, c * W2 + sub * P : c * W2 + (sub + 1) * P],
                            in_=zt_ps[:],
                        )
                u_ps = [
                    psm_p.tile(
                        [P, W2], f32, space="PSUM", tag="misc", name=f"ups{g}_{j}"
                    )
                    for j in range(Cj)
                ]
                for j in range(Cj):
                    for c in range(Ci):
                        fi = c * Cj + j
                        nc.tensor.matmul(
                            out=u_ps[j][:],
                            lhsT=win_sb[:, fi * P : (fi + 1) * P],
                            rhs=zt_sb[:, c * W2 : (c + 1) * W2],
                            start=(c == 0),
                            stop=(c == Ci - 1),
                        )
                X = act_p.tile([P, Cr * W2], bf16, tag="X", name=f"X{g}_0")
                for j in range(Cj):
                    emit_silu(X[:, j * W2 : (j + 1) * W2], u_ps[j][:])
                for l in range(NH):
                    v_ps = [
                        psm_p.tile(
                            [P, W2], f32, space="PSUM", tag="misc", name=f"vps{g}_{l}_{j}"
                        )
                        for j in range(Cr)
                    ]
                    for j in range(Cr):
                        for i in range(Cr):
                            fi = ((l * 2 + 0) * Cr + i) * Cr + j
                            nc.tensor.matmul(
                                out=v_ps[j][:],
                                lhsT=wres_sb[:, fi * P : (fi + 1) * P],
                                rhs=X[:, i * W2 : (i + 1) * W2],
                                start=(i == 0),
                                stop=(i == Cr - 1),
                            )
                    u1 = act_p.tile([P, Cr * W2], bf16, tag="u1", name=f"u1_{g}_{l}")
                    for j in range(Cr):
                        emit_silu(u1[:, j * W2 : (j + 1) * W2], v_ps[j][:])
                    w_ps = [
                        psm_p.tile(
                            [P, W2], f32, space="PSUM", tag="misc", name=f"wps{g}_{l}_{j}"
                        )
                        for j in range(Cr)
                    ]
                    for j in range(Cr):
                        for i in range(Cr):
                            fi = ((l * 2 + 1) * Cr + i) * Cr + j
                            nc.tensor.matmul(
                                out=w_ps[j][:],
                                lhsT=wres_sb[:, fi * P : (fi + 1) * P],
                                rhs=u1[:, i * W2 : (i + 1) * W2],
                                start=(i == 0),
                                stop=(i == Cr - 1),
                            )
                    Y = act_p.tile([P, Cr * W2], bf16, tag="y", name=f"Y{g}_{l}")
                    for j in range(Cr):
                        emit_silu(Y[:, j * W2 : (j + 1) * W2], w_ps[j][:])
                    Xn = act_p.tile([P, Cr * W2], bf16, tag="X", name=f"X{g}_{l + 1}")
                    nc.vector.scalar_tensor_tensor(
                        out=Xn[:],
                        in0=Y[:],
                        scalar=GAMMA[l],
                        in1=X[:],
                        op0=mybir.AluOpType.mult,
                        op1=mybir.AluOpType.add,
                    )
                    X = Xn
                o_t = out_p.tile([P, Cj * W2], f32, tag="out")
                nc.scalar.mul(out=o_t[:], in_=X[:], mul=float(C3))
                for j in range(Cj):
                    nc.sync.dma_start(
                        out=d_out[:, (j * t_atom + 2 * g) * P : (j * t_atom + 2 * g + 2) * P],
                        in_=o_t[:, j * W2 : (j + 1) * W2],
                    )
    nc.compile()
    return nc


def _get_nc(t_atom, K):
    key = (t_atom, K)
    if key not in _NC_CACHE:
        _NC_CACHE[key] = _build_nc(t_atom, K)
    return _NC_CACHE[key]


# ----------------------------------------------------------------------------
# Entry point
# ----------------------------------------------------------------------------

def kernel(h, m, basis_rad, idx_atom, W_rbf, W_in, res_W1, res_W2):
    from concourse.bass_utils import run_bass_kernel_spmd

    m = np.asarray(m, dtype=np.float32)
    basis_rad = np.asarray(basis_rad, dtype=np.float32)
    idx = np.asarray(idx_atom).astype(np.int64)
    W_rbf = np.asarray(W_rbf, dtype=np.float32)
    W_in = np.asarray(W_in, dtype=np.float32)
    res_W1 = np.asarray(res_W1, dtype=np.float32)
    res_W2 = np.asarray(res_W2, dtype=np.float32)
    n_atoms = np.asarray(h).shape[0]

    layout = _pack_layout(idx, n_atoms, N_CORES, T_ATOM)
    in_maps = _build_in_maps(
        m, basis_rad, layout, W_rbf, W_in, res_W1, res_W2, N_CORES, T_ATOM
    )
    nc = _get_nc(T_ATOM, layout["K"])

    trace = os.environ.get("KERNEL_TRACE", "0") == "1"
    res = run_bass_kernel_spmd(
        nc, in_maps, core_ids=list(range(N_CORES)), trace=trace
    )
    if trace and res.exec_time_ns is not None:
        print(f"HW exec time: {res.exec_time_ns} ns", file=sys.stderr)
        kernel.last_exec_time_ns = res.exec_time_ns
    kernel.last_results = res
    return _unpack_output(res.results, layout, n_atoms, N_CORES, T_ATOM)


# revision 21
# speedup vs baseline: 3.4728x; 1.3558x over previous
"""Trainium2 Bass kernel for GemNet AtomUpdateBlock (gnn_message_passing).

Computation (per reference):
    bases = basis_rad @ W_rbf              # [E, De]
    x     = m * bases                      # [E, De]
    z     = segment_sum(x, idx_atom, A)    # [A, De]
    x     = silu(z @ W_in)                 # [A, Da]
    3x residual: x = (x + silu(silu(x W1) W2)) / sqrt(2)

Distribution strategy: shard EDGES BY DESTINATION ATOM. The host bins the
atoms into 8 cores x T_ATOM tiles of <=128 atoms (balanced by edge count),
sorts/pads each tile's edges into K 128-edge groups, and each core computes
the segment-sum + atom MLP for its own atoms only. No collective needed;
outputs are disjoint atom slices.

Per 128-edge tile on device (bf16 matmuls, f32 PSUM):
    PE:  bases_psum = basis_radT_tile.T @ W_rbf     (K=16, N=512)
    ACT: evacuate bases cols [0,EVAC) psum->sbuf bf16
    DVE: x[0:EVAC] = bases_sb * m (bf16 2x), x[EVAC:] = psum * m
    PE:  z[a,:] += S.T @ x  (one matmul, N=512; S = one-hot scatter matrix
         precomputed host-side, DMA'd bf16, loaded as PE weights)
Epilogue per PAIR of 128-atom tiles: z evac -> 4 PE transposes (f32) to
feature-major zT -> bf16 MLP matmuls N=256, silu on ACT, skip-adds as one
fused DVE scalar_tensor_tensor per layer with host-folded sqrt2 scaling.
"""

import math
import os
import sys

import numpy as np
import ml_dtypes

BF16 = ml_dtypes.bfloat16

P = 128
N_CORES = 8
DE, DA, DR, NH = 512, 256, 16, 3
T_ATOM = 20  # atom tiles per core (each up to 128 atoms); must be even
INV_SQRT_2 = 0.7071067811865476

_NC_CACHE = {}
SILU_NATIVE = True


# ----------------------------------------------------------------------------
# Host-side packing
# ----------------------------------------------------------------------------

def _pack_layout(idx, n_atoms, n_cores, t_atom):
    E = idx.shape[0]
    n_bins = n_cores * t_atom
    counts = np.bincount(idx, minlength=n_atoms)

    order = np.argsort(-counts, kind="stable")
    n_rounds = math.ceil(n_atoms / n_bins)
    pad = n_rounds * n_bins - n_atoms
    padded = np.concatenate([order, np.full(pad, -1, dtype=order.dtype)])
    grid = padded.reshape(n_rounds, n_bins)
    grid[1::2] = grid[1::2, ::-1]  # snake-deal: balances edges and atoms
    bin_of_atom = np.empty(n_atoms, dtype=np.int64)
    slot_of_atom = np.empty(n_atoms, dtype=np.int64)
    valid = grid >= 0
    bin_idx = np.broadcast_to(np.arange(n_bins), grid.shape)
    round_idx = np.broadcast_to(np.arange(n_rounds)[:, None], grid.shape)
    bin_of_atom[grid[valid]] = bin_idx[valid]
    slot_of_atom[grid[valid]] = round_idx[valid]
    assert np.bincount(bin_of_atom, minlength=n_bins).max() <= P

    ebin = bin_of_atom[idx]
    eslot = slot_of_atom[idx]
    eorder = np.argsort(ebin * (P + 1) + eslot, kind="stable")
    ebin_sorted = ebin[eorder]
    bin_counts = np.bincount(ebin_sorted, minlength=n_bins)
    K = max(1, math.ceil(bin_counts.max() / P))
    bin_starts = np.zeros(n_bins + 1, dtype=np.int64)
    np.cumsum(bin_counts, out=bin_starts[1:])
    pos_in_bin = np.arange(E) - bin_starts[ebin_sorted]

    core_of_bin = np.arange(n_bins) // t_atom
    tile_of_bin = np.arange(n_bins) % t_atom
    return dict(
        K=K,
        eorder=eorder,
        core_of_edge=core_of_bin[ebin_sorted],
        flat_slot=tile_of_bin[ebin_sorted] * (K * P) + pos_in_bin,
        rel_of_edge=eslot[eorder].astype(np.int64),
        bin_of_atom=bin_of_atom,
        slot_of_atom=slot_of_atom,
        core_of_bin=core_of_bin,
        tile_of_bin=tile_of_bin,
    )


def _pack_weights(W_rbf, W_in, res_W1, res_W2):
    Ci, Cj = DE // P, DA // P
    Cr = DA // P
    win = W_in.reshape(Ci, P, Cj, P).transpose(1, 0, 2, 3).reshape(P, Ci * Cj * P)
    blocks = []
    c = INV_SQRT_2
    for l in range(NH):
        w1 = (res_W1[l] * (c ** l)).astype(np.float32)
        w2 = res_W2[l].astype(np.float32)
        for W in (w1, w2):
            blocks.append(
                W.reshape(Cr, P, Cr, P).transpose(1, 0, 2, 3).reshape(P, Cr * Cr * P)
            )
    wres = np.concatenate(blocks, axis=1)
    return (
        np.ascontiguousarray(W_rbf, dtype=BF16),
        np.ascontiguousarray(win, dtype=BF16),
        np.ascontiguousarray(wres, dtype=BF16),
    )


def _build_in_maps(m, basis_rad, layout, W_rbf, W_in, res_W1, res_W2, n_cores, t_atom):
    K = layout["K"]
    cap = t_atom * K * P
    ncols = t_atom * K
    eorder = layout["eorder"]
    core_of_edge = layout["core_of_edge"]
    flat_slot = layout["flat_slot"]
    rel = layout["rel_of_edge"]

    wrbf, win, wres = _pack_weights(W_rbf, W_in, res_W1, res_W2)
    m_src = m[eorder]
    bas_src = basis_rad[eorder]

    in_maps = []
    for c in range(n_cores):
        sel = core_of_edge == c
        fs = flat_slot[sel]
        m_pack = np.zeros((cap, DE), dtype=BF16)
        m_pack[fs] = m_src[sel].astype(BF16)
        # partition-major: m2[p, col*DE + d] = m_pack[col*P + p, d] so each
        # partition's per-atom-tile DMA read is fully contiguous
        m_pack = np.ascontiguousarray(
            m_pack.reshape(ncols, P, DE).transpose(1, 0, 2).reshape(P, ncols * DE)
        )
        basT = np.zeros((DR, cap), dtype=BF16)
        basT[:, fs] = bas_src[sel].T.astype(BF16)
        rel_flat = np.full(cap, -1, dtype=np.int64)
        rel_flat[fs] = rel[sel]
        rel2 = rel_flat.reshape(ncols, P).T  # [p, col]
        s_host = (rel2[:, :, None] == np.arange(P)[None, None, :]).astype(BF16)
        in_maps.append(
            dict(
                m_pack=m_pack,
                basT=np.ascontiguousarray(basT),
                s_hot=np.ascontiguousarray(s_host.reshape(P, ncols * P)),
                wrbf=wrbf,
                win=win,
                wres=wres,
            )
        )
    return in_maps


def _unpack_output(results, layout, n_atoms, n_cores, t_atom):
    Cj = DA // P
    out = np.zeros((n_atoms, DA), dtype=np.float32)
    core_of_atom = layout["core_of_bin"][layout["bin_of_atom"]]
    row_of_atom = (
        layout["tile_of_bin"][layout["bin_of_atom"]] * P + layout["slot_of_atom"]
    )
    for c in range(n_cores):
        x = results[c]["out"].reshape(P, Cj, t_atom, P)
        x_core = x.transpose(2, 3, 1, 0).reshape(t_atom * P, DA)
        mask = core_of_atom == c
        out[mask] = x_core[row_of_atom[mask]]
    return out


# ----------------------------------------------------------------------------
# Bass kernel builder
# ----------------------------------------------------------------------------

def _build_nc(t_atom, K):
    import concourse.mybir as mybir
    import concourse.tile as tile
    from concourse import bacc
    from concourse.masks import make_identity

    f32 = mybir.dt.float32
    bf16 = mybir.dt.bfloat16
    Ci, Cj = DE // P, DA // P
    Cr = DA // P
    cap = t_atom * K * P
    ncols = t_atom * K
    C3 = INV_SQRT_2 ** NH
    GAMMA = [float((1.0 / INV_SQRT_2) ** l) for l in range(NH)]
    assert t_atom % 2 == 0
    n_pairs = t_atom // 2
    W2 = 2 * P  # atoms per epilogue pair

    nc = bacc.Bacc(
        "TRN2",
        target_bir_lowering=False,
        debug=False,
        enable_asserts=False,
        num_devices=N_CORES,
    )
    d_m = nc.dram_tensor("m_pack", [P, ncols * DE], bf16, kind="ExternalInput")
    d_basT = nc.dram_tensor("basT", [DR, cap], bf16, kind="ExternalInput")
    d_s = nc.dram_tensor("s_hot", [P, ncols * P], bf16, kind="ExternalInput")
    d_wrbf = nc.dram_tensor("wrbf", [DR, DE], bf16, kind="ExternalInput")
    d_win = nc.dram_tensor("win", [P, Ci * Cj * P], bf16, kind="ExternalInput")
    d_wres = nc.dram_tensor(
        "wres", [P, NH * 2 * Cr * Cr * P], bf16, kind="ExternalInput"
    )
    d_out = nc.dram_tensor("out", [P, Cj * t_atom * P], f32, kind="ExternalOutput")

    with tile.TileContext(nc) as tc:
        with (
            tc.tile_pool(name="const", bufs=1) as const_p,
            tc.tile_pool(name="bas", bufs=2) as bas_p,
            tc.tile_pool(name="m", bufs=2) as m_p,
            tc.tile_pool(name="x", bufs=4) as x_p,
            tc.tile_pool(name="s", bufs=2) as s_p,
            tc.tile_pool(name="zsb", bufs=2) as zsb_p,
            tc.tile_pool(name="ztsb", bufs=2) as ztsb_p,
            tc.tile_pool(name="act", bufs=3) as act_p,
            tc.tile_pool(name="outp", bufs=2) as out_p,
            tc.tile_pool(name="ps_bases", bufs=3, space="PSUM") as psb_p,
            tc.tile_pool(name="ps_z", bufs=2, space="PSUM") as psz_p,
            tc.tile_pool(name="ps_misc", bufs=3, space="PSUM") as psm_p,
        ):
            _ctr = [0]

            def emit_silu(out_ap, in_ps_ap):
                if SILU_NATIVE:
                    nc.scalar.activation(
                        out=out_ap, in_=in_ps_ap,
                        func=mybir.ActivationFunctionType.Silu,
                    )
                else:
                    _ctr[0] += 1
                    sg = act_p.tile([P, W2], f32, tag="sig", name=f"sig{_ctr[0]}")
                    nc.scalar.activation(
                        out=sg[:], in_=in_ps_ap,
                        func=mybir.ActivationFunctionType.Sigmoid,
                    )
                    nc.vector.tensor_tensor(
                        out=out_ap, in0=in_ps_ap, in1=sg[:],
                        op=mybir.AluOpType.mult,
                    )

            # Resident constants
            wrbf_sb = const_p.tile([DR, DE], bf16, tag="wrbf")
            nc.sync.dma_start(out=wrbf_sb[:], in_=d_wrbf[:])
            win_sb = const_p.tile([P, Ci * Cj * P], bf16, tag="win")
            nc.sync.dma_start(out=win_sb[:], in_=d_win[:])
            wres_sb = const_p.tile([P, NH * 2 * Cr * Cr * P], bf16, tag="wres")
            nc.sync.dma_start(out=wres_sb[:], in_=d_wres[:])
            ident = const_p.tile([P, P], f32, tag="ident")
            make_identity(nc, ident[:])

            for g in range(n_pairs):
                z_sub = []
                for sub in range(2):
                    t = 2 * g + sub
                    bas_sb = bas_p.tile([DR, K * P], bf16, tag="bas")
                    nc.sync.dma_start(
                        out=bas_sb[:], in_=d_basT[:, t * K * P : (t + 1) * K * P]
                    )
                    # whole atom tile's m and S in one DMA each
                    m_t = m_p.tile([P, K * DE], bf16, tag="m")
                    nc.sync.dma_start(
                        out=m_t[:], in_=d_m[:, t * K * DE : (t + 1) * K * DE]
                    )
                    s_t = s_p.tile([P, K * P], bf16, tag="s")
                    nc.sync.dma_start(
                        out=s_t[:], in_=d_s[:, t * K * P : (t + 1) * K * P]
                    )
                    z_ps = psz_p.tile(
                        [P, DE], f32, space="PSUM", tag="z", name=f"zps{t}"
                    )
                    z_sub.append(z_ps)
                    for k in range(K):
                        b_ps = psb_p.tile([P, DE], f32, space="PSUM", tag="bases")
                        nc.tensor.matmul(
                            out=b_ps[:],
                            lhsT=bas_sb[:, k * P : (k + 1) * P],
                            rhs=wrbf_sb[:],
                            start=True,
                            stop=True,
                        )
                        x_t = x_p.tile([P, DE], bf16, tag="x")
                        nc.vector.tensor_tensor(
                            out=x_t[:],
                            in0=b_ps[:],
                            in1=m_t[:, k * DE : (k + 1) * DE],
                            op=mybir.AluOpType.mult,
                        )
                        nc.tensor.matmul(
                            out=z_ps[:],
                            lhsT=s_t[:, k * P : (k + 1) * P],
                            rhs=x_t[:],
                            start=(k == 0),
                            stop=(k == K - 1),
                        )
                # ---- paired epilogue (256 atoms, feature-major, bf16) ----
                zt_sb = ztsb_p.tile([P, Ci * W2], bf16, tag="ztsb")
                for sub in range(2):
                    t = 2 * g + sub
                    z_sb = zsb_p.tile([P, DE], f32, tag="zsb", name=f"zsb{t}")
                    nc.scalar.copy(out=z_sb[:], in_=z_sub[sub][:])
                    for c in range(Ci):
                        zt_ps = psm_p.tile(
                            [P, P], f32, space="PSUM", tag="misc", name=f"ztp{t}_{c}"
                        )
                        nc.tensor.transpose(
                            out=zt_ps[:],
                            in_=z_sb[:, c * P : (c + 1) * P],
                            identity=ident[:],
                        )
                        nc.scalar.copy(
                            out=zt_sb[:, c * W2 + sub * P : c * W2 + (sub + 1) * P],
                            in_=zt_ps[:],
                        )
                u_ps = [
                    psm_p.tile(
                        [P, W2], f32, space="PSUM", tag="misc", name=f"ups{g}_{j}"
                    )
                    for j in range(Cj)
                ]
                for j in range(Cj):
                    for c in range(Ci):
                        fi = c * Cj + j
                        nc.tensor.matmul(
                            out=u_ps[j][:],
                            lhsT=win_sb[:, fi * P : (fi + 1) * P],
                            rhs=zt_sb[:, c * W2 : (c + 1) * W2],
                            start=(c == 0),
                            stop=(c == Ci - 1),
                        )
                X = act_p.tile([P, Cr * W2], bf16, tag="X", name=f"X{g}_0")
                for j in range(Cj):
                    emit_silu(X[:, j * W2 : (j + 1) * W2], u_ps[j][:])
                for l in range(NH):
                    v_ps = [
                        psm_p.tile(
                            [P, W2], f32, space="PSUM", tag="misc", name=f"vps{g}_{l}_{j}"
                        )
                        for j in range(Cr)
                    ]
                    for j in range(Cr):
                        for i in range(Cr):
                            fi = ((l * 2 + 0) * Cr + i) * Cr + j
                            nc.tensor.matmul(
                                out=v_ps[j][:],
                                lhsT=wres_sb[:, fi * P : (fi + 1) * P],
                                rhs=X[:, i * W2 : (i + 1) * W2],
                                start=(i == 0),
                                stop=(i == Cr - 1),
                            )
                    u1 = act_p.tile([P, Cr * W2], bf16, tag="u1", name=f"u1_{g}_{l}")
                    for j in range(Cr):
                        emit_silu(u1[:, j * W2 : (j + 1) * W2], v_ps[j][:])
                    w_ps = [
                        psm_p.tile(
                            [P, W2], f32, space="PSUM", tag="misc", name=f"wps{g}_{l}_{j}"
                        )
                        for j in range(Cr)
                    ]
                    for j in range(Cr):
                        for i in range(Cr):
                            fi = ((l * 2 + 1) * Cr + i) * Cr + j
                            nc.tensor.matmul(
                                out=w_ps[j][:],
                                lhsT=wres_sb[:, fi * P : (fi + 1) * P],
                                rhs=u1[:, i * W2 : (i + 1) * W2],
                                start=(i == 0),
                                stop=(i == Cr - 1),
                            )
                    Y = act_p.tile([P, Cr * W2], bf16, tag="y", name=f"Y{g}_{l}")
                    for j in range(Cr):
                        emit_silu(Y[:, j * W2 : (j + 1) * W2], w_ps[j][:])
                    Xn = act_p.tile([P, Cr * W2], bf16, tag="X", name=f"X{g}_{l + 1}")
                    nc.vector.scalar_tensor_tensor(
                        out=Xn[:],
                        in0=Y[:],
                        scalar=GAMMA[l],
                        in1=X[:],
                        op0=mybir.AluOpType.mult,
                        op1=mybir.AluOpType.add,
                    )
                    X = Xn
                o_t = out_p.tile([P, Cj * W2], f32, tag="out")
                nc.scalar.mul(out=o_t[:], in_=X[:], mul=float(C3))
                for j in range(Cj):
                    nc.sync.dma_start(
                        out=d_out[:, (j * t_atom + 2 * g) * P : (j * t_atom + 2 * g + 2) * P],
                        in_=o_t[:, j * W2 : (j + 1) * W2],
                    )
    nc.compile()
    return nc


def _get_nc(t_atom, K):
    key = (t_atom, K)
    if key not in _NC_CACHE:
        _NC_CACHE[key] = _build_nc(t_atom, K)
    return _NC_CACHE[key]


# ----------------------------------------------------------------------------
# Entry point
# ----------------------------------------------------------------------------

def kernel(h, m, basis_rad, idx_atom, W_rbf, W_in, res_W1, res_W2):
    from concourse.bass_utils import run_bass_kernel_spmd

    m = np.asarray(m, dtype=np.float32)
    basis_rad = np.asarray(basis_rad, dtype=np.float32)
    idx = np.asarray(idx_atom).astype(np.int64)
    W_rbf = np.asarray(W_rbf, dtype=np.float32)
    W_in = np.asarray(W_in, dtype=np.float32)
    res_W1 = np.asarray(res_W1, dtype=np.float32)
    res_W2 = np.asarray(res_W2, dtype=np.float32)
    n_atoms = np.asarray(h).shape[0]

    layout = _pack_layout(idx, n_atoms, N_CORES, T_ATOM)
    in_maps = _build_in_maps(
        m, basis_rad, layout, W_rbf, W_in, res_W1, res_W2, N_CORES, T_ATOM
    )
    nc = _get_nc(T_ATOM, layout["K"])

    trace = os.environ.get("KERNEL_TRACE", "0") == "1"
    res = run_bass_kernel_spmd(
        nc, in_maps, core_ids=list(range(N_CORES)), trace=trace
    )
    if trace and res.exec_time_ns is not None:
        print(f"HW exec time: {res.exec_time_ns} ns", file=sys.stderr)
        kernel.last_exec_time_ns = res.exec_time_ns
    kernel.last_results = res
    return _unpack_output(res.results, layout, n_atoms, N_CORES, T_ATOM)
